# revision 2
# baseline (speedup 1.0000x reference)
"""Trainium2 Bass kernel for LocalSpatioTemporalPooling.

Reference computation (per sample n):
  x: (C=256, T=30, H=64, W=44) fp32
  feats[c,t,s] = mean over the (8,44) spatial stripe s of frame t    # 352-elem mean
  scores[t,s] = || feats[:,t,s] ||_2  (clip eps)                     # reduce over C
  top-2 frames per stripe by score; output[s*C + c] = mean of the 2 selected feats

Sharding: pure data parallel — one sample per NeuronCore (N=8 = n_cores).

Kernel layout per core:
  x viewed as (C, T*S*352); groups of 352 are contiguous in HBM.
  - Stream 12 tiles (2 c-blocks x 6 t-chunks of 5 frames), 7.04MB each, contiguous
    per partition; DVE tensor_reduce sums each 352-group -> feats (128, 240) per
    c-block, stored stripe-major (free index = s*30 + t).
  - ACT squares feats, PE column-sum (ones lhsT) -> sumsq (1, 240) in PSUM.
    Ordering by sumsq == ordering by the reference score (monotonic transform).
  - Tiny DMA scatters sumsq to (8, 30) (stripe per partition); DVE Max8 gives the
    top-8 per stripe in descending order; weight w = (score >= top[1]) * 0.5/352.
  - w -> (1, 240), PE broadcast to (128, 240), multiply + strided reduce over t
    -> (128, 8) per c-block; PE transpose -> (8, 128); assemble (8, 256); one DMA out.
"""

import numpy as np
from contextlib import ExitStack

import concourse.bass as bass
import concourse.tile as tile
import concourse.mybir as mybir
from concourse import bacc
from concourse.bass_utils import run_bass_kernel_spmd
from concourse.masks import make_identity

N, C, T, H, W = 8, 256, 30, 64, 44
S = 8                 # stripes
SH = H // S           # 8 rows per stripe
GROUP = SH * W        # 352 elements per (c, t, s) group
TCH = 5               # frames per tile
NCH = T // TCH        # 6 chunks
CB = C // 128         # 2 channel blocks
FRAME = H * W         # 2816
WSCALE = 0.5 / GROUP  # top-2 mean of stripe means

_F32 = mybir.dt.float32


def _kernel_body(ctx, tc, nc, x, out):
    const_pool = ctx.enter_context(tc.tile_pool(name="const", bufs=1))
    in_pool = ctx.enter_context(tc.tile_pool(name="inp", bufs=3))
    feat_pool = ctx.enter_context(tc.tile_pool(name="feat", bufs=1))
    small_pool = ctx.enter_context(tc.tile_pool(name="small", bufs=1))
    psum_pool = ctx.enter_context(tc.tile_pool(name="psum", bufs=1, space="PSUM"))

    ones_col = const_pool.tile([128, 1], _F32)
    nc.vector.memset(ones_col[:], 1.0)
    ones_row = const_pool.tile([1, 128], _F32)
    nc.vector.memset(ones_row[:], 1.0)
    identity = const_pool.tile([128, 128], _F32)
    make_identity(nc, identity[:])

    feats = [feat_pool.tile([128, T * S], _F32, tag=f"feats{cb}", name=f"feats{cb}") for cb in range(CB)]

    # ---- main streaming reduction: x -> feats (stripe-major) ----
    for cb in range(CB):
        for j in range(NCH):
            tl = in_pool.tile([128, TCH * S * GROUP], _F32)
            nc.sync.dma_start(
                tl[:], x[cb * 128:(cb + 1) * 128, j * TCH * FRAME:(j + 1) * TCH * FRAME]
            )
            in4 = tl[:].rearrange("p (t s w) -> p t s w", t=TCH, s=S)
            outap = feats[cb][:].rearrange("p (s t) -> p t s", s=S)[
                :, j * TCH:(j + 1) * TCH, :
            ]
            nc.vector.tensor_reduce(
                outap, in4, axis=mybir.AxisListType.X, op=mybir.AluOpType.add
            )

    # ---- scores: sumsq over channels via ACT square + PE column-sum ----
    ss_psum = psum_pool.tile([1, T * S], _F32, tag="ss")
    for cb in range(CB):
        sq = small_pool.tile([128, T * S], _F32, tag=f"sq{cb}")
        nc.scalar.activation(sq[:], feats[cb][:], mybir.ActivationFunctionType.Square)
        nc.tensor.matmul(
            ss_psum[:], lhsT=ones_col[:], rhs=sq[:], start=(cb == 0), stop=(cb == CB - 1)
        )
    ss_sb = small_pool.tile([1, T * S], _F32)
    nc.scalar.copy(ss_sb[:], ss_psum[:])

    # ---- per-stripe top-2 -> weights ----
    sc8 = small_pool.tile([S, T], _F32)
    nc.sync.dma_start(sc8[:], ss_sb[0:1, :].rearrange("p (s t) -> p s t", s=S))
    top8 = small_pool.tile([S, 8], _F32)
    nc.vector.max(top8[:], sc8[:])
    w8 = small_pool.tile([S, T], _F32)
    nc.vector.tensor_scalar(
        w8[:], sc8[:], top8[:, 1:2], WSCALE,
        op0=mybir.AluOpType.is_ge, op1=mybir.AluOpType.mult,
    )
    wv = small_pool.tile([1, T * S], _F32)
    nc.sync.dma_start(wv[0:1, :].rearrange("p (s t) -> p s t", s=S), w8[:])
    wb_psum = psum_pool.tile([128, T * S], _F32, tag="wb")
    nc.tensor.matmul(wb_psum[:], lhsT=ones_row[:], rhs=wv[:], start=True, stop=True)

    # ---- weighted frame mean + output assembly ----
    outsb = small_pool.tile([S, C], _F32)
    for cb in range(CB):
        prod = small_pool.tile([128, T * S], _F32, tag=f"prod{cb}")
        nc.vector.tensor_mul(prod[:], feats[cb][:], wb_psum[:])
        oblk = small_pool.tile([128, S], _F32, tag=f"oblk{cb}")
        nc.vector.tensor_reduce(
            oblk[:], prod[:].rearrange("p (s t) -> p s t", s=S),
            axis=mybir.AxisListType.X, op=mybir.AluOpType.add,
        )
        tr_psum = psum_pool.tile([S, 128], _F32, tag=f"tr{cb}")
        nc.tensor.transpose(tr_psum[:], oblk[:], identity[:])
        nc.vector.tensor_copy(outsb[:, cb * 128:(cb + 1) * 128], tr_psum[:])
    nc.sync.dma_start(out[:, :], outsb[:])


_NC_CACHE = {}


def _get_nc():
    if "nc" not in _NC_CACHE:
        nc = bacc.Bacc("TRN2", target_bir_lowering=False, debug=False)
        x = nc.dram_tensor("x", [C, T * FRAME], _F32, kind="ExternalInput").ap()
        out = nc.dram_tensor("out", [S, C], _F32, kind="ExternalOutput").ap()
        with tile.TileContext(nc) as tc:
            with ExitStack() as ctx:
                _kernel_body(ctx, tc, nc, x, out)
        nc.compile()
        _NC_CACHE["nc"] = nc
    return _NC_CACHE["nc"]


def kernel(x):
    x = np.asarray(x, dtype=np.float32)
    assert x.shape == (N, C, T, H, W), x.shape
    nc = _get_nc()
    in_maps = [{"x": np.ascontiguousarray(x[i]).reshape(C, T * FRAME)} for i in range(N)]
    res = run_bass_kernel_spmd(nc, in_maps, list(range(N)))
    return np.stack([res.results[i]["out"].reshape(S * C) for i in range(N)])
